# revision 28
# baseline (speedup 1.0000x reference)
"""Trainium2 Bass kernel for nn_NeuroScribe: CNN feature extractor + DMP integrator.

Strategy (per core, 512 samples, pure data-parallel across 8 cores):
  - Host folds L_w into fc_w (only 7 FC outputs needed: goal, w[5], tau),
    parity-packs samples (M = (out_ch, sample-parity) = 128), and builds the
    conv1 im2col (incl. a ones-row for the bias) in fp16.
  - conv1: 48 groups of 4 MMs (K=71, N=512) into one 4-bank PSUM tile
    (pe/po halves); relu+pool fused into eviction: ACT relu-evicts the po
    half, DVE scalar_tensor_tensor fuses relu(pe)+o_s, GPSIMD (mostly)
    folds the last pool level into h1.
  - conv2: two K=64 parity streams interleaved per-MM so they run in
    different PE row-groups concurrently (2x). 2-bank PSUM tiles (1 quad,
    10 MMs each). conv2 bias is NOT added on-chip: the eviction computes
    relu(x+b)-b via max(x,-b)+relu(po+b), and the constant offset is
    folded into the fc bias on the host.
  - fc: w7 [128,7] slices as stationary operand -> psum [7, 512]; 24
    accumulating MMs; PE transposes (identity trick) deliver g7 [128,4,7].
  - DMP: closed form. B_Z = A_Z/4 => critically damped: the 2x2 transition
    is lam*I + N with N nilpotent. x_t = d^t (geometric). All recurrences
    become tensor_tensor_scan ops; psi/fx evaluated for all t at once.
        y_t = lam^t y0 + t lam^(t-1) q1 + u S_t,  q1 = u(12.5 y0 + u)
        C_{t+1} = lam C_t + beta_t ; S_{t+1} = lam S_t + C_t
        beta_t = u (156.25 goal + fx_t)
"""
import os
import numpy as np

import concourse.bass as bass
import concourse.bacc as bacc
import concourse.mybir as mybir
from concourse import tile
from concourse import bass_utils

f32 = mybir.dt.float32
f16 = mybir.dt.float16
i32 = mybir.dt.int32
AF = mybir.ActivationFunctionType
ALU = mybir.AluOpType

N_CORES = 8
B = 4096
BC = B // N_CORES          # 512 samples per core
B2 = BC // 2               # 256 parity pairs
T = 101
NT = 100                   # scan steps
DT = 0.01
N_RBF = 5
_C = np.exp(-np.linspace(0.0, 1.0, N_RBF)).astype(np.float32)
_SIG2 = ((N_RBF ** 1.5) / _C).astype(np.float32)

L1 = 384                   # conv1 positions
Q1 = 96                    # pooled positions after pool1
Q2 = 24                    # pooled positions after pool2
K1 = 71                    # conv1 contraction (2 parities x 5ci x 7k + bias)
NPOS_STRIP = 64            # conv1 positions per X1 strip
N_STRIPS = L1 // NPOS_STRIP

# knobs: which conv pool-level-2 adds go to DVE vs GPSIMD
C1_L2_DVE_EVERY = 10 ** 9  # every Nth conv1 quad's l2-add goes to DVE
C2_L2_GS_EVERY = 4         # every Nth conv2 quad's l2-add goes to GPSIMD


def _dmp_prep(nc, dp):
    """g7-independent DMP constants; emitted early so they overlap conv."""
    st = {}
    st["ones"] = dp.tile([128, T], f32, name="ones")
    nc.vector.memset(st["ones"][:], 1.0)
    st["zeros"] = dp.tile([128, NT], f32, name="zeros")
    nc.vector.memset(st["zeros"][:], 0.0)
    tio = dp.tile([128, T], i32)
    nc.gpsimd.iota(tio[:], [[1, T]], base=0, channel_multiplier=0)
    st["tful"] = dp.tile([128, T], f32, name="tful")
    nc.vector.tensor_copy(st["tful"][:], tio[:])
    # only 10 physical [128,4,T] buffers; later phases alias tiles whose
    # earlier occupant is dead by then (saves ~10KB/partition of SBUF).
    for nm in ("xs", "lamt", "lamf_all", "x2", "den"):
        st[nm] = dp.tile([128, 4, T], f32, name=nm)
    st["psi"] = [dp.tile([128, 4, T], f32, name=f"psi{j}")
                 for j in range(N_RBF)]
    st["num"] = st["x2"]       # x2 dead once psi args built
    st["fx2"] = st["psi"][0]   # psi dead after the num chain
    st["beta"] = st["psi"][1]
    st["Cs"] = st["psi"][2]
    st["Ss"] = st["psi"][3]
    st["yout"] = st["psi"][4]
    st["sc"] = dp.tile([128, 12, 4], f32, name="sc")
    st["gsc"] = dp.tile([128, T], f32, name="gsc")
    return st


def _emit_dmp(nc, dp, st, g7, y0t, outd):
    ones = st["ones"]
    zeros = st["zeros"]
    tful = st["tful"]
    xs = st["xs"]
    lamt = st["lamt"]
    lamf_all = st["lamf_all"]
    Cs = st["Cs"]
    Ss = st["Ss"]
    beta = st["beta"]
    num = st["num"]
    den = st["den"]
    fx2 = st["fx2"]
    yout = st["yout"]
    psi = st["psi"]

    def R2(t):
        return t.rearrange("p a b -> p (a b)")

    # ---- per-sample scalars, batched over the 4 sample chunks ----
    sc = st["sc"]
    tau = g7[:, :, 6]
    goal = g7[:, :, 0]
    u = sc[:, 0, :]
    lam = sc[:, 1, :]
    dg = sc[:, 2, :]
    kgy = sc[:, 3, :]
    q1l = sc[:, 4, :]
    bsc = sc[:, 5, :]
    bct = sc[:, 6, :]
    t0 = sc[:, 7, :]
    t1 = sc[:, 8, :]
    rl = sc[:, 9, :]
    t2 = sc[:, 10, :]
    nc.vector.tensor_scalar_mul(u, tau, DT)
    nc.vector.tensor_scalar(lam, tau, -0.125, 1.0, ALU.mult, ALU.add)
    nc.vector.tensor_scalar(dg, tau, -0.01, 1.0, ALU.mult, ALU.add)
    nc.vector.tensor_sub(kgy, goal, y0t[:])
    nc.vector.scalar_tensor_tensor(t0, y0t[:], 12.5, u, ALU.mult, ALU.add)
    nc.vector.tensor_mul(t1, u, t0)
    nc.vector.reciprocal(rl, lam)
    nc.vector.tensor_mul(q1l, t1, rl)
    nc.vector.tensor_mul(bsc, u, kgy)
    nc.vector.tensor_mul(t2, tau, goal)
    nc.vector.tensor_scalar_mul(bct, t2, 1.5625)

    for c in range(4):
        lamf = lamf_all[:, c, :]
        nc.vector.tensor_scalar_mul(lamf, ones[:], sc[:, 1, c:c + 1])
        dgf = beta[:, c, :]   # scratch
        nc.vector.tensor_scalar_mul(dgf, ones[:], sc[:, 2, c:c + 1])
        nc.vector.memset(xs[:, c, 0:1], 1.0)
        nc.vector.tensor_tensor_scan(
            xs[:, c, 1:T], dgf[:, 0:NT], zeros[:], 1.0,
            ALU.mult, ALU.add)
        nc.vector.memset(lamt[:, c, 0:1], 1.0)
        nc.vector.tensor_tensor_scan(
            lamt[:, c, 1:T], lamf[:, 0:NT], zeros[:], 1.0,
            ALU.mult, ALU.add)

    # psi_j = exp(a_j x^2 + b_j x + d_j): one shared Square, then per-j
    # affine (DVE/GS) + Exp (ACT) — shorter ACT chain than Square+Exp per j.
    x2 = st["x2"]
    nc.scalar.activation(R2(x2), R2(xs), AF.Square)
    for j in range(N_RBF):
        a_j = float(-0.5 / _SIG2[j])
        b_j = float(_C[j] / _SIG2[j])
        d_j = float(-0.5 * _C[j] * _C[j] / _SIG2[j])
        nc.vector.tensor_scalar(R2(psi[j]), R2(x2), a_j, d_j,
                                ALU.mult, ALU.add)
        nc.vector.scalar_tensor_tensor(R2(psi[j]), R2(xs), b_j,
                                       R2(psi[j]), ALU.mult, ALU.add)
        nc.scalar.activation(R2(psi[j]), R2(psi[j]), AF.Exp)
    nc.gpsimd.tensor_add(R2(den), R2(psi[0]), R2(psi[1]))
    nc.gpsimd.tensor_add(R2(fx2), R2(psi[2]), R2(psi[3]))
    nc.gpsimd.tensor_add(R2(den), R2(den), R2(fx2))
    nc.gpsimd.tensor_add(R2(den), R2(den), R2(psi[4]))
    nc.vector.reciprocal(R2(den), R2(den))

    gsc = st["gsc"]
    for c in range(4):
        ncol = num[:, c, :]
        if c < 2:
            nc.vector.tensor_scalar_mul(ncol, psi[0][:, c, :],
                                        g7[:, c, 1:2])
            for j in range(1, N_RBF):
                nc.vector.scalar_tensor_tensor(
                    ncol, psi[j][:, c, :], g7[:, c, 1 + j:2 + j],
                    ncol, ALU.mult, ALU.add)
        else:
            nc.gpsimd.tensor_scalar_mul(ncol, psi[0][:, c, :],
                                        g7[:, c, 1:2])
            for j in range(1, N_RBF):
                nc.gpsimd.tensor_scalar_mul(gsc[:], psi[j][:, c, :],
                                            g7[:, c, 1 + j:2 + j])
                nc.gpsimd.tensor_add(ncol, ncol, gsc[:])
    nc.vector.tensor_mul(R2(fx2), R2(num), R2(den))
    nc.vector.tensor_mul(R2(fx2), R2(fx2), R2(xs))

    for c in range(4):
        uc = sc[:, 0, c:c + 1]
        q1c = sc[:, 4, c:c + 1]
        bscc = sc[:, 5, c:c + 1]
        bctc = sc[:, 6, c:c + 1]
        y0c = y0t[:, c:c + 1]
        nc.vector.tensor_scalar(beta[:, c, :], fx2[:, c, :],
                                bscc, bctc, ALU.mult, ALU.add)
        nc.vector.memset(Cs[:, c, 0:1], 0.0)
        nc.vector.tensor_tensor_scan(
            Cs[:, c, 1:T], lamf_all[:, c, 0:NT],
            beta[:, c, 0:NT], 0.0, ALU.mult, ALU.add)
        nc.vector.memset(Ss[:, c, 0:1], 0.0)
        nc.vector.tensor_tensor_scan(
            Ss[:, c, 1:T], lamf_all[:, c, 0:NT],
            Cs[:, c, 0:NT], 0.0, ALU.mult, ALU.add)
        # y = lamt*(y0 + t*q1l) + u*S
        a1 = num[:, c, :]
        nc.vector.tensor_scalar(a1, tful[:], q1c, y0c, ALU.mult, ALU.add)
        b1 = den[:, c, :]
        nc.gpsimd.tensor_mul(b1, lamt[:, c, :], a1)
        nc.vector.scalar_tensor_tensor(
            yout[:, c, :], Ss[:, c, :], uc, b1, ALU.mult, ALU.add)
        nc.sync.dma_start(outd[0:64, c, :], yout[0:64, c, :])
        nc.scalar.dma_start(outd[64:128, c, :], yout[64:128, c, :])


def build_program(weights, repeat=1):
    nc = bacc.Bacc(None, target_bir_lowering=False, debug=True)

    # strip-major so each strip is one contiguous DRAM region; every DMA
    # instruction stays <= 64 descriptors (else it degrades to one engine).
    x1d = nc.dram_tensor("x1", [N_STRIPS, K1, NPOS_STRIP, B2], f16,
                         kind="ExternalInput")
    y0d = nc.dram_tensor("y0c", [128, 4], f32, kind="ExternalInput")
    outd = nc.dram_tensor("out", [128, 4, T], f32, kind="ExternalOutput")

    w1d = nc.inline_tensor(weights["W1p"], "W1p")       # [K1, 128] f16
    w2d = nc.inline_tensor(weights["W2t"], "W2t")       # [128, 5, 128] f16
    w7d = nc.inline_tensor(weights["W7t"], "W7t")       # [128, 24, 7] f16
    b2d = nc.inline_tensor(weights["b2c"], "b2c")       # [128, 1] f32
    nb2d = nc.inline_tensor(weights["nb2"], "nb2")      # [128, 1] f32
    b7d = nc.inline_tensor(weights["b7rep"], "b7rep")   # [128, 4, 7] f32
    eyed = nc.inline_tensor(weights["eye7"], "eye7")    # [7, 7] f32

    with tile.TileContext(nc) as tc:
      for _rep in range(repeat):
        with tc.tile_pool(name="const", bufs=1) as cp, \
             tc.tile_pool(name="dmp", bufs=1) as dp, \
             tc.tile_pool(name="x1p", bufs=3) as xp:
            # strip prefetch first so the PE isn't blocked behind the
            # (latency-tolerant) weight loads on the two DMA rings.
            x1tiles = []

            def load_strip(s):
                x1t = xp.tile([128, NPOS_STRIP, B2], f16, tag="x1t",
                              name=f"x1t{s}")
                nc.sync.dma_start(x1t[0:36], x1d[s, 0:36])
                nc.scalar.dma_start(x1t[36:K1], x1d[s, 36:K1])
                x1tiles.append(x1t)

            load_strip(0)
            w1t = cp.tile([K1, 128], f16)
            nc.sync.dma_start(w1t[0:36, :], w1d[0:36, :])
            nc.scalar.dma_start(w1t[36:K1, :], w1d[36:K1, :])
            load_strip(1)
            w2t = cp.tile([128, 5, 128], f16)
            nc.sync.dma_start(w2t[0:64], w2d[0:64])
            nc.scalar.dma_start(w2t[64:128], w2d[64:128])
            b2t = cp.tile([128, 1], f32)
            nc.sync.dma_start(b2t[0:64], b2d[0:64])
            nc.scalar.dma_start(b2t[64:128], b2d[64:128])
            nb2t = cp.tile([128, 1], f32)
            nc.sync.dma_start(nb2t[0:64], nb2d[0:64])
            nc.scalar.dma_start(nb2t[64:128], nb2d[64:128])
            load_strip(2)
            w7t = cp.tile([128, 24, 7], f16)
            nc.sync.dma_start(w7t[0:64], w7d[0:64])
            nc.scalar.dma_start(w7t[64:128], w7d[64:128])
            b7t = cp.tile([128, 4, 7], f32)
            nc.sync.dma_start(b7t[0:64], b7d[0:64])
            nc.scalar.dma_start(b7t[64:128], b7d[64:128])
            eye7 = cp.tile([7, 7], f32)
            nc.sync.dma_start(eye7[:], eyed[:])
            y0t = cp.tile([128, 4], f32)
            nc.sync.dma_start(y0t[0:64], y0d[0:64])
            nc.scalar.dma_start(y0t[64:128], y0d[64:128])
            dmp_st = _dmp_prep(nc, dp)

            # h1: [128=(par*64+ch), 100=(q in -2..97), 256] fp16, zero-padded
            h1 = cp.tile([128, Q1 + 4, B2], f16)
            nc.vector.memset(h1[:, 0:2, :], 0.0)
            nc.vector.memset(h1[:, Q1 + 2:Q1 + 4, :], 0.0)
            # h2p: [128=co2, 2=parity, 24=q4, 256] fp16
            h2p = cp.tile([128, 2, Q2, B2], f16, name="h2p")
            g7 = cp.tile([128, 4, 7], f32, name="g7")

            # ---------------- conv1 + conv2, interleaved ----------------
            # Quad-granular 2-bank PSUM tiles from one shared pool with
            # bufs=4 (8 banks): depth-4 pipelining hides the ~1.4us
            # eviction latency so the PE streams continuously and the HAM
            # clock-gate stays at full rate.
            # conv1 quad q1 (of 96): 2 MMs (pos pairs 2q1, 2q1+1).
            # conv2 quad q (of 24): 10 MMs per parity, parities
            # interleaved per-MM -> concurrent PE row-groups.
            with tc.tile_pool(name="ps", bufs=4, space="PSUM") as ps, \
                 tc.tile_pool(name="stg", bufs=3) as stp, \
                 tc.tile_pool(name="st2", bufs=3) as st2:

                def conv1_quad(q1):
                    s, lq = divmod(q1, NPOS_STRIP // 4)
                    if lq == 0 and s >= 3:
                        load_strip(s)
                    x1t = x1tiles[s]
                    pst = ps.tile([128, 2, 2, B2], f32, tag="ps",
                                  name=f"c1p{q1}")
                    for e in range(2):
                        m = lq * 2 + e
                        nc.tensor.matmul(
                            pst[:, e, :, :], w1t[:, :],
                            x1t[0:K1, 2 * m:2 * m + 2, :],
                            start=True, stop=True)
                    o_s = stp.tile([128, 2, B2], f16, tag="o_s",
                                   name=f"c1os{q1}")
                    nc.scalar.activation(o_s[:], pst[:, 1, :, :], AF.Relu)
                    l1t = stp.tile([128, 2, B2], f16, tag="l1t",
                                   name=f"c1l{q1}")
                    nc.vector.scalar_tensor_tensor(
                        l1t[:], pst[:, 0, :, :], 0.0, o_s[:],
                        ALU.max, ALU.add)
                    # l2: h1[2+q1] = l1t[0]+l1t[1]
                    eng = (nc.vector if (q1 % C1_L2_DVE_EVERY
                                         == C1_L2_DVE_EVERY - 1)
                           else nc.gpsimd)
                    eng.tensor_add(h1[:, 2 + q1, :],
                                   l1t[:, 0, :], l1t[:, 1, :])

                def conv2_quad(q):
                    pstA = ps.tile([128, 2, 2, B2], f32, tag="ps",
                                   name=f"c2a{q}")
                    pstB = ps.tile([128, 2, 2, B2], f32, tag="ps",
                                   name=f"c2b{q}")
                    for k in range(5):
                        for i in range(2):
                            pp = 2 * q + i
                            nc.tensor.matmul(
                                pstA[:, i, :, :], w2t[0:64, k, :],
                                h1[0:64, 2 * pp + k:2 * pp + k + 2, :],
                                start=(k == 0), stop=(k == 4))
                            nc.tensor.matmul(
                                pstB[:, i, :, :], w2t[64:128, k, :],
                                h1[64:128, 2 * pp + k:2 * pp + k + 2, :],
                                start=(k == 0), stop=(k == 4))
                    for par, pst2 in ((0, pstA), (1, pstB)):
                        o2 = st2.tile([128, 2, B2], f16, tag="o2",
                                      name=f"c2o{par}_{q}")
                        nc.scalar.activation(o2[:], pst2[:, 1, :, :],
                                             AF.Relu, bias=b2t[:, 0:1])
                        l2t = st2.tile([128, 2, B2], f16, tag="l2t",
                                       name=f"c2l{par}_{q}")
                        nc.vector.scalar_tensor_tensor(
                            l2t[:], pst2[:, 0, :, :], nb2t[:, 0:1], o2[:],
                            ALU.max, ALU.add)
                        eng = (nc.gpsimd if (q % C2_L2_GS_EVERY == 0)
                               else nc.vector)
                        eng.tensor_add(h2p[:, par, q, :],
                                       l2t[:, 0, :], l2t[:, 1, :])

                # conv2 quad q needs conv1 quads through 4q+5
                emitted = 0
                for q1 in range(96):
                    conv1_quad(q1)
                    while emitted < Q2 and 4 * emitted + 5 <= q1:
                        conv2_quad(emitted)
                        emitted += 1
                while emitted < Q2:
                    conv2_quad(emitted)
                    emitted += 1

            # ---------------- fc: g7 = [samples, 7] ----------------
            with tc.tile_pool(name="psg", bufs=1, space="PSUM") as psg, \
                 tc.tile_pool(name="fst", bufs=1) as fst:
                pg = psg.tile([7, 2, B2], f32, tag="pg")
                for q4 in range(Q2):
                    nc.tensor.matmul(
                        pg[:], w7t[:, q4, :], h2p[:, :, q4, :],
                        start=(q4 == 0), stop=(q4 == Q2 - 1))
                g7s = fst.tile([7, 2, B2], f32)
                nc.vector.tensor_copy(g7s[:], pg[:])
                pgT = psg.tile([128, 4, 7], f32, tag="pgT")
                for c in range(4):
                    par, half = c // 2, c % 2
                    nc.tensor.transpose(
                        pgT[:, c, :],
                        g7s[:, par, half * 128:half * 128 + 128],
                        eye7[:])
                nc.vector.tensor_add(g7[:], pgT[:], b7t[:])

            # ---------------- DMP closed form ----------------
            _emit_dmp(nc, dp, dmp_st, g7, y0t, outd)

    nc.compile()
    return nc


# --------------------------------------------------------------------------
# host-side prep
# --------------------------------------------------------------------------

def prep_weights(conv1_w, conv1_b, conv2_w, conv2_b, fc_w, fc_b, L_w, L_b):
    W1p = np.zeros((K1, 128), np.float32)
    for h in range(2):
        W1p[h * 35:(h + 1) * 35, h * 64:h * 64 + 64] = \
            conv1_w.reshape(64, 35).T
        W1p[70, h * 64:h * 64 + 64] = conv1_b
    W2t = np.zeros((128, 5, 128), np.float32)
    for k in range(5):
        W2t[0:64, k, :] = conv2_w[:, :, k].T * 0.25
        W2t[64:128, k, :] = conv2_w[:, :, k].T * 0.25
    Wfc7 = np.concatenate(
        [fc_w[0:6].astype(np.float64),
         (L_w.astype(np.float64) @ fc_w.astype(np.float64))], axis=0)
    W7t = np.zeros((128, Q2, 7), np.float32)
    for j in range(7):
        W7t[:, :, j] = Wfc7[j].reshape(128, Q2) * 0.25
    b7 = np.concatenate(
        [fc_b[0:6].astype(np.float64),
         L_w.astype(np.float64) @ fc_b.astype(np.float64)
         + L_b.astype(np.float64)])
    # the on-chip h2p is sum(relu(conv2+b2)) - 2*b2 per (channel, quad);
    # fold the constant back in through the fc bias.
    b2_64 = conv2_b.astype(np.float64)
    corr = 2.0 * np.einsum(
        "cqj,c->j",
        Wfc7.reshape(7, 128, Q2).transpose(1, 2, 0) * 0.25, b2_64)
    b7 = b7 + corr
    b7rep = np.broadcast_to(
        b7.astype(np.float32)[None, None, :], (128, 4, 7)).copy()
    return {
        "W1p": W1p.astype(np.float16),
        "W2t": W2t.astype(np.float16),
        "W7t": W7t.astype(np.float16),
        "b2c": conv2_b.reshape(128, 1).astype(np.float32),
        "nb2": (-conv2_b).reshape(128, 1).astype(np.float32),
        "b7rep": np.ascontiguousarray(b7rep),
        "eye7": np.eye(7, dtype=np.float32),
    }


def prep_core_inputs(input_full, y0_full, core):
    base = core * BC
    inp = input_full[base:base + BC]
    inp_pad = np.zeros((BC, 5, L1 + 6), np.float32)
    inp_pad[:, :, 3:3 + L1] = inp
    X1 = np.empty((K1, L1, B2), np.float16)
    for h in range(2):
        samp = inp_pad[2 * np.arange(B2) + h]
        for ci in range(5):
            for k in range(7):
                X1[h * 35 + ci * 7 + k] = \
                    samp[:, ci, k:k + L1].T.astype(np.float16)
    X1[70] = 1.0
    # strip-major: [N_STRIPS, K1, NPOS_STRIP, B2], each strip contiguous
    X1 = np.ascontiguousarray(
        X1.reshape(K1, N_STRIPS, NPOS_STRIP, B2).transpose(1, 0, 2, 3))
    y0c = y0_full[base:base + BC]
    perm = np.concatenate([np.arange(0, BC, 2), np.arange(1, BC, 2)])
    y0dev = y0c[perm].astype(np.float32).reshape(4, 128).T.copy()
    return {"x1": X1, "y0c": np.ascontiguousarray(y0dev)}, perm


_CACHE = {}
LAST_RESULTS = None


def kernel(input, y0, conv1_w, conv1_b, conv2_w, conv2_b, fc_w, fc_b, L_w, L_b):
    key = "nc"
    if key not in _CACHE:
        weights = prep_weights(conv1_w, conv1_b, conv2_w, conv2_b,
                               fc_w, fc_b, L_w, L_b)
        _CACHE[key] = build_program(
            weights, repeat=int(os.environ.get("KERNEL_REPEAT", "1")))
    nc = _CACHE[key]

    in_maps = []
    perms = []
    for core in range(N_CORES):
        im, perm = prep_core_inputs(input, y0, core)
        in_maps.append(im)
        perms.append(perm)

    trace = bool(int(os.environ.get("KERNEL_TRACE", "0")))
    res = bass_utils.run_bass_kernel_spmd(
        nc, in_maps, core_ids=list(range(N_CORES)), trace=trace)
    global LAST_RESULTS
    LAST_RESULTS = res

    out = np.empty((B, T, 1), np.float32)
    for core in range(N_CORES):
        ydev = res.results[core]["out"].transpose(1, 0, 2).reshape(BC, T)
        base = core * BC
        out[base + perms[core], :, 0] = ydev
    return out


# revision 35
# speedup vs baseline: 1.0593x; 1.0593x over previous
"""Trainium2 Bass kernel for nn_NeuroScribe: CNN feature extractor + DMP integrator.

Strategy (per core, 512 samples, pure data-parallel across 8 cores):
  - Host folds L_w into fc_w (only 7 FC outputs needed: goal, w[5], tau),
    parity-packs samples (M = (out_ch, sample-parity) = 128), and builds the
    conv1 im2col (incl. a ones-row for the bias) in fp16.
  - conv1: 48 groups of 4 MMs (K=71, N=512) into one 4-bank PSUM tile
    (pe/po halves); relu+pool fused into eviction: ACT relu-evicts the po
    half, DVE scalar_tensor_tensor fuses relu(pe)+o_s, GPSIMD (mostly)
    folds the last pool level into h1.
  - conv2: two K=64 parity streams interleaved per-MM so they run in
    different PE row-groups concurrently (2x). 2-bank PSUM tiles (1 quad,
    10 MMs each). conv2 bias is NOT added on-chip: the eviction computes
    relu(x+b)-b via max(x,-b)+relu(po+b), and the constant offset is
    folded into the fc bias on the host.
  - fc: w7 [128,7] slices as stationary operand -> psum [7, 512]; 24
    accumulating MMs; PE transposes (identity trick) deliver g7 [128,4,7].
  - DMP: closed form. B_Z = A_Z/4 => critically damped: the 2x2 transition
    is lam*I + N with N nilpotent. x_t = d^t (geometric). All recurrences
    become tensor_tensor_scan ops; psi/fx evaluated for all t at once.
        y_t = lam^t y0 + t lam^(t-1) q1 + u S_t,  q1 = u(12.5 y0 + u)
        C_{t+1} = lam C_t + beta_t ; S_{t+1} = lam S_t + C_t
        beta_t = u (156.25 goal + fx_t)
"""
import os
import numpy as np

import concourse.bass as bass
import concourse.bacc as bacc
import concourse.mybir as mybir
from concourse import tile
from concourse import bass_utils

f32 = mybir.dt.float32
f16 = mybir.dt.float16
i32 = mybir.dt.int32
AF = mybir.ActivationFunctionType
ALU = mybir.AluOpType

N_CORES = 8
B = 4096
BC = B // N_CORES          # 512 samples per core
B2 = BC // 2               # 256 parity pairs
T = 101
NT = 100                   # scan steps
DT = 0.01
N_RBF = 5
_C = np.exp(-np.linspace(0.0, 1.0, N_RBF)).astype(np.float32)
_SIG2 = ((N_RBF ** 1.5) / _C).astype(np.float32)

L1 = 384                   # conv1 positions
Q1 = 96                    # pooled positions after pool1
Q2 = 24                    # pooled positions after pool2
K1 = 71                    # conv1 contraction (2 parities x 5ci x 7k + bias)
NPOS_STRIP = 64            # conv1 positions per X1 strip
N_STRIPS = L1 // NPOS_STRIP

# knobs: which conv pool-level-2 adds go to DVE vs GPSIMD
C1_L2_DVE_EVERY = 10 ** 9  # every Nth conv1 quad's l2-add goes to DVE
C2_L2_GS_EVERY = 4         # every Nth conv2 quad's l2-add goes to GPSIMD


def _dmp_prep(nc, dp):
    """g7-independent DMP constants; emitted early so they overlap conv."""
    st = {}
    st["ones"] = dp.tile([128, T], f32, name="ones")
    nc.vector.memset(st["ones"][:], 1.0)
    st["zeros"] = dp.tile([128, NT], f32, name="zeros")
    nc.vector.memset(st["zeros"][:], 0.0)
    tio = dp.tile([128, T], i32)
    nc.gpsimd.iota(tio[:], [[1, T]], base=0, channel_multiplier=0)
    st["tful"] = dp.tile([128, T], f32, name="tful")
    nc.vector.tensor_copy(st["tful"][:], tio[:])
    # only 10 physical [128,4,T] buffers; later phases alias tiles whose
    # earlier occupant is dead by then (saves ~10KB/partition of SBUF).
    for nm in ("xs", "lamt", "lamf_all", "x2", "den"):
        st[nm] = dp.tile([128, 4, T], f32, name=nm)
    st["psi"] = [dp.tile([128, 4, T], f32, name=f"psi{j}")
                 for j in range(N_RBF)]
    st["num"] = st["x2"]       # x2 dead once psi args built
    st["fx2"] = st["psi"][0]   # psi dead after the num chain
    st["beta"] = st["psi"][1]
    st["Cs"] = st["psi"][2]
    st["Ss"] = st["psi"][3]
    st["yout"] = st["psi"][4]
    st["sc"] = dp.tile([128, 12, 4], f32, name="sc")
    return st


def _emit_dmp(nc, dp, st, g7, y0t, outd):
    ones = st["ones"]
    zeros = st["zeros"]
    tful = st["tful"]
    xs = st["xs"]
    lamt = st["lamt"]
    lamf_all = st["lamf_all"]
    Cs = st["Cs"]
    Ss = st["Ss"]
    beta = st["beta"]
    num = st["num"]
    den = st["den"]
    fx2 = st["fx2"]
    yout = st["yout"]
    psi = st["psi"]

    def R2(t):
        return t.rearrange("p a b -> p (a b)")

    # ---- per-sample scalars, batched over the 4 sample chunks ----
    sc = st["sc"]
    tau = g7[:, :, 6]
    goal = g7[:, :, 0]
    u = sc[:, 0, :]
    lam = sc[:, 1, :]
    dg = sc[:, 2, :]
    kgy = sc[:, 3, :]
    q1l = sc[:, 4, :]
    bsc = sc[:, 5, :]
    bct = sc[:, 6, :]
    t0 = sc[:, 7, :]
    t1 = sc[:, 8, :]
    rl = sc[:, 9, :]
    t2 = sc[:, 10, :]
    nc.vector.tensor_scalar_mul(u, tau, DT)
    nc.vector.tensor_scalar(lam, tau, -0.125, 1.0, ALU.mult, ALU.add)
    nc.vector.tensor_scalar(dg, tau, -0.01, 1.0, ALU.mult, ALU.add)
    nc.vector.tensor_sub(kgy, goal, y0t[:])
    nc.vector.scalar_tensor_tensor(t0, y0t[:], 12.5, u, ALU.mult, ALU.add)
    nc.vector.tensor_mul(t1, u, t0)
    nc.vector.reciprocal(rl, lam)
    nc.vector.tensor_mul(q1l, t1, rl)
    nc.vector.tensor_mul(bsc, u, kgy)
    nc.vector.tensor_mul(t2, tau, goal)
    nc.vector.tensor_scalar_mul(bct, t2, 1.5625)

    for c in range(4):
        lamf = lamf_all[:, c, :]
        nc.vector.tensor_scalar_mul(lamf, ones[:], sc[:, 1, c:c + 1])
        dgf = beta[:, c, :]   # scratch
        nc.vector.tensor_scalar_mul(dgf, ones[:], sc[:, 2, c:c + 1])
        nc.vector.memset(xs[:, c, 0:1], 1.0)
        nc.vector.tensor_tensor_scan(
            xs[:, c, 1:T], dgf[:, 0:NT], zeros[:], 1.0,
            ALU.mult, ALU.add)
        nc.vector.memset(lamt[:, c, 0:1], 1.0)
        nc.vector.tensor_tensor_scan(
            lamt[:, c, 1:T], lamf[:, 0:NT], zeros[:], 1.0,
            ALU.mult, ALU.add)

    # psi_j = exp(a_j x^2 + b_j x + d_j): one shared Square, then per-j
    # affine (DVE/GS) + Exp (ACT) — shorter ACT chain than Square+Exp per j.
    x2 = st["x2"]
    nc.scalar.activation(R2(x2), R2(xs), AF.Square)
    for j in range(N_RBF):
        a_j = float(-0.5 / _SIG2[j])
        b_j = float(_C[j] / _SIG2[j])
        d_j = float(-0.5 * _C[j] * _C[j] / _SIG2[j])
        nc.vector.tensor_scalar(R2(psi[j]), R2(x2), a_j, d_j,
                                ALU.mult, ALU.add)
        nc.vector.scalar_tensor_tensor(R2(psi[j]), R2(xs), b_j,
                                       R2(psi[j]), ALU.mult, ALU.add)
        nc.scalar.activation(R2(psi[j]), R2(psi[j]), AF.Exp)
    nc.gpsimd.tensor_add(R2(den), R2(psi[0]), R2(psi[1]))
    nc.gpsimd.tensor_add(R2(fx2), R2(psi[2]), R2(psi[3]))
    nc.gpsimd.tensor_add(R2(den), R2(den), R2(fx2))
    nc.gpsimd.tensor_add(R2(den), R2(den), R2(psi[4]))
    nc.vector.reciprocal(R2(den), R2(den))

    for c in range(4):
        ncol = num[:, c, :]
        nc.vector.tensor_scalar_mul(ncol, psi[0][:, c, :], g7[:, c, 1:2])
        for j in range(1, N_RBF):
            nc.vector.scalar_tensor_tensor(
                ncol, psi[j][:, c, :], g7[:, c, 1 + j:2 + j],
                ncol, ALU.mult, ALU.add)
    nc.vector.tensor_mul(R2(fx2), R2(num), R2(den))
    nc.vector.tensor_mul(R2(fx2), R2(fx2), R2(xs))

    for c in range(4):
        uc = sc[:, 0, c:c + 1]
        q1c = sc[:, 4, c:c + 1]
        bscc = sc[:, 5, c:c + 1]
        bctc = sc[:, 6, c:c + 1]
        y0c = y0t[:, c:c + 1]
        nc.vector.tensor_scalar(beta[:, c, :], fx2[:, c, :],
                                bscc, bctc, ALU.mult, ALU.add)
        nc.vector.memset(Cs[:, c, 0:1], 0.0)
        nc.vector.tensor_tensor_scan(
            Cs[:, c, 1:T], lamf_all[:, c, 0:NT],
            beta[:, c, 0:NT], 0.0, ALU.mult, ALU.add)
        nc.vector.memset(Ss[:, c, 0:1], 0.0)
        nc.vector.tensor_tensor_scan(
            Ss[:, c, 1:T], lamf_all[:, c, 0:NT],
            Cs[:, c, 0:NT], 0.0, ALU.mult, ALU.add)
        # y = lamt*(y0 + t*q1l) + u*S
        a1 = num[:, c, :]
        nc.vector.tensor_scalar(a1, tful[:], q1c, y0c, ALU.mult, ALU.add)
        b1 = den[:, c, :]
        nc.gpsimd.tensor_mul(b1, lamt[:, c, :], a1)
        nc.vector.scalar_tensor_tensor(
            yout[:, c, :], Ss[:, c, :], uc, b1, ALU.mult, ALU.add)
        nc.sync.dma_start(outd[0:64, c, :], yout[0:64, c, :])
        nc.scalar.dma_start(outd[64:128, c, :], yout[64:128, c, :])


def build_program(weights, repeat=1):
    nc = bacc.Bacc(None, target_bir_lowering=False, debug=True)

    # strip-major so each strip is one contiguous DRAM region; every DMA
    # instruction stays <= 64 descriptors (else it degrades to one engine).
    x1d = nc.dram_tensor("x1", [N_STRIPS, K1, NPOS_STRIP, B2], f16,
                         kind="ExternalInput")
    y0d = nc.dram_tensor("y0c", [128, 4], f32, kind="ExternalInput")
    outd = nc.dram_tensor("out", [128, 4, T], f32, kind="ExternalOutput")

    w1d = nc.inline_tensor(weights["W1p"], "W1p")       # [K1, 128] f16
    w2d = nc.inline_tensor(weights["W2t"], "W2t")       # [128, 5, 128] f16
    w7d = nc.inline_tensor(weights["W7t"], "W7t")       # [128, 24, 7] f16
    b2d = nc.inline_tensor(weights["b2c"], "b2c")       # [128, 1] f32
    nb2d = nc.inline_tensor(weights["nb2"], "nb2")      # [128, 1] f32
    b7d = nc.inline_tensor(weights["b7rep"], "b7rep")   # [128, 4, 7] f32
    eyed = nc.inline_tensor(weights["eye7"], "eye7")    # [7, 7] f32

    with tile.TileContext(nc) as tc:
      for _rep in range(repeat):
        with tc.tile_pool(name="const", bufs=1) as cp, \
             tc.tile_pool(name="dmp", bufs=1) as dp, \
             tc.tile_pool(name="x1p", bufs=3) as xp:
            # strip prefetch first so the PE isn't blocked behind the
            # (latency-tolerant) weight loads on the two DMA rings.
            x1tiles = []

            def load_strip(s):
                x1t = xp.tile([128, NPOS_STRIP, B2], f16, tag="x1t",
                              name=f"x1t{s}")
                nc.sync.dma_start(x1t[0:36], x1d[s, 0:36])
                nc.scalar.dma_start(x1t[36:K1], x1d[s, 36:K1])
                x1tiles.append(x1t)

            w1t = cp.tile([K1, 128], f16)
            nc.sync.dma_start(w1t[0:36, :], w1d[0:36, :])
            nc.scalar.dma_start(w1t[36:K1, :], w1d[36:K1, :])
            load_strip(0)
            load_strip(1)
            w2t = cp.tile([128, 5, 128], f16)
            nc.sync.dma_start(w2t[0:64], w2d[0:64])
            nc.scalar.dma_start(w2t[64:128], w2d[64:128])
            b2t = cp.tile([128, 1], f32)
            nc.sync.dma_start(b2t[0:64], b2d[0:64])
            nc.scalar.dma_start(b2t[64:128], b2d[64:128])
            nb2t = cp.tile([128, 1], f32)
            nc.sync.dma_start(nb2t[0:64], nb2d[0:64])
            nc.scalar.dma_start(nb2t[64:128], nb2d[64:128])
            load_strip(2)
            w7t = cp.tile([128, 24, 7], f16)
            nc.sync.dma_start(w7t[0:64], w7d[0:64])
            nc.scalar.dma_start(w7t[64:128], w7d[64:128])
            b7t = cp.tile([128, 4, 7], f32)
            nc.sync.dma_start(b7t[0:64], b7d[0:64])
            nc.scalar.dma_start(b7t[64:128], b7d[64:128])
            eye7 = cp.tile([7, 7], f32)
            nc.sync.dma_start(eye7[:], eyed[:])
            y0t = cp.tile([128, 4], f32)
            nc.sync.dma_start(y0t[0:64], y0d[0:64])
            nc.scalar.dma_start(y0t[64:128], y0d[64:128])
            dmp_st = _dmp_prep(nc, dp)

            # h1: [128=(par*64+ch), 100=(q in -2..97), 256] fp16, zero-padded
            h1 = cp.tile([128, Q1 + 4, B2], f16)
            nc.vector.memset(h1[:, 0:2, :], 0.0)
            nc.vector.memset(h1[:, Q1 + 2:Q1 + 4, :], 0.0)
            # h2p: [128=co2, 2=parity, 24=q4, 256] fp16
            h2p = cp.tile([128, 2, Q2, B2], f16, name="h2p")
            g7 = cp.tile([128, 4, 7], f32, name="g7")

            # ---------------- conv1 + conv2, interleaved ----------------
            # Quad-granular 2-bank PSUM tiles from one shared pool with
            # bufs=4 (8 banks): depth-4 pipelining hides the ~1.4us
            # eviction latency so the PE streams continuously and the HAM
            # clock-gate stays at full rate.
            # conv1 quad q1 (of 96): 2 MMs (pos pairs 2q1, 2q1+1).
            # conv2 quad q (of 24): 10 MMs per parity, parities
            # interleaved per-MM -> concurrent PE row-groups.
            with tc.tile_pool(name="ps", bufs=4, space="PSUM") as ps, \
                 tc.tile_pool(name="stg", bufs=3) as stp, \
                 tc.tile_pool(name="st2", bufs=3) as st2:

                def conv1_quad(q1):
                    s, lq = divmod(q1, NPOS_STRIP // 4)
                    if lq == 0 and s >= 3:
                        load_strip(s)
                    x1t = x1tiles[s]
                    pst = ps.tile([128, 2, 2, B2], f32, tag="ps",
                                  name=f"c1p{q1}")
                    for e in range(2):
                        m = lq * 2 + e
                        nc.tensor.matmul(
                            pst[:, e, :, :], w1t[:, :],
                            x1t[0:K1, 2 * m:2 * m + 2, :],
                            start=True, stop=True)
                    o_s = stp.tile([128, 2, B2], f16, tag="o_s",
                                   name=f"c1os{q1}")
                    nc.scalar.activation(o_s[:], pst[:, 1, :, :], AF.Relu)
                    l1t = stp.tile([128, 2, B2], f16, tag="l1t",
                                   name=f"c1l{q1}")
                    nc.vector.scalar_tensor_tensor(
                        l1t[:], pst[:, 0, :, :], 0.0, o_s[:],
                        ALU.max, ALU.add)
                    # l2: h1[2+q1] = l1t[0]+l1t[1]
                    eng = (nc.vector if (q1 % C1_L2_DVE_EVERY
                                         == C1_L2_DVE_EVERY - 1)
                           else nc.gpsimd)
                    eng.tensor_add(h1[:, 2 + q1, :],
                                   l1t[:, 0, :], l1t[:, 1, :])

                def conv2_quad(q):
                    pstA = ps.tile([128, 2, 2, B2], f32, tag="ps",
                                   name=f"c2a{q}")
                    pstB = ps.tile([128, 2, 2, B2], f32, tag="ps",
                                   name=f"c2b{q}")
                    for k in range(5):
                        for i in range(2):
                            pp = 2 * q + i
                            nc.tensor.matmul(
                                pstA[:, i, :, :], w2t[0:64, k, :],
                                h1[0:64, 2 * pp + k:2 * pp + k + 2, :],
                                start=(k == 0), stop=(k == 4))
                            nc.tensor.matmul(
                                pstB[:, i, :, :], w2t[64:128, k, :],
                                h1[64:128, 2 * pp + k:2 * pp + k + 2, :],
                                start=(k == 0), stop=(k == 4))
                    for par, pst2 in ((0, pstA), (1, pstB)):
                        o2 = st2.tile([128, 2, B2], f16, tag="o2",
                                      name=f"c2o{par}_{q}")
                        nc.scalar.activation(o2[:], pst2[:, 1, :, :],
                                             AF.Relu, bias=b2t[:, 0:1])
                        l2t = st2.tile([128, 2, B2], f16, tag="l2t",
                                       name=f"c2l{par}_{q}")
                        nc.vector.scalar_tensor_tensor(
                            l2t[:], pst2[:, 0, :, :], nb2t[:, 0:1], o2[:],
                            ALU.max, ALU.add)
                        eng = (nc.gpsimd if (q % C2_L2_GS_EVERY == 0)
                               else nc.vector)
                        eng.tensor_add(h2p[:, par, q, :],
                                       l2t[:, 0, :], l2t[:, 1, :])

                # conv2 quad q needs conv1 quads through 4q+5
                emitted = 0
                for q1 in range(96):
                    conv1_quad(q1)
                    while emitted < Q2 and 4 * emitted + 5 <= q1:
                        conv2_quad(emitted)
                        emitted += 1
                while emitted < Q2:
                    conv2_quad(emitted)
                    emitted += 1

            # ---------------- fc: g7 = [samples, 7] ----------------
            with tc.tile_pool(name="psg", bufs=1, space="PSUM") as psg, \
                 tc.tile_pool(name="fst", bufs=1) as fst:
                pg = psg.tile([7, 2, B2], f32, tag="pg")
                for q4 in range(Q2):
                    nc.tensor.matmul(
                        pg[:], w7t[:, q4, :], h2p[:, :, q4, :],
                        start=(q4 == 0), stop=(q4 == Q2 - 1))
                g7s = fst.tile([7, 2, B2], f32)
                nc.vector.tensor_copy(g7s[:], pg[:])
                pgT = psg.tile([128, 4, 7], f32, tag="pgT")
                for c in range(4):
                    par, half = c // 2, c % 2
                    nc.tensor.transpose(
                        pgT[:, c, :],
                        g7s[:, par, half * 128:half * 128 + 128],
                        eye7[:])
                nc.vector.tensor_add(g7[:], pgT[:], b7t[:])

            # ---------------- DMP closed form ----------------
            _emit_dmp(nc, dp, dmp_st, g7, y0t, outd)

    nc.compile()
    return nc


# --------------------------------------------------------------------------
# host-side prep
# --------------------------------------------------------------------------

def prep_weights(conv1_w, conv1_b, conv2_w, conv2_b, fc_w, fc_b, L_w, L_b):
    W1p = np.zeros((K1, 128), np.float32)
    for h in range(2):
        W1p[h * 35:(h + 1) * 35, h * 64:h * 64 + 64] = \
            conv1_w.reshape(64, 35).T
        W1p[70, h * 64:h * 64 + 64] = conv1_b
    W2t = np.zeros((128, 5, 128), np.float32)
    for k in range(5):
        W2t[0:64, k, :] = conv2_w[:, :, k].T * 0.25
        W2t[64:128, k, :] = conv2_w[:, :, k].T * 0.25
    Wfc7 = np.concatenate(
        [fc_w[0:6].astype(np.float64),
         (L_w.astype(np.float64) @ fc_w.astype(np.float64))], axis=0)
    W7t = np.zeros((128, Q2, 7), np.float32)
    for j in range(7):
        W7t[:, :, j] = Wfc7[j].reshape(128, Q2) * 0.25
    b7 = np.concatenate(
        [fc_b[0:6].astype(np.float64),
         L_w.astype(np.float64) @ fc_b.astype(np.float64)
         + L_b.astype(np.float64)])
    # the on-chip h2p is sum(relu(conv2+b2)) - 2*b2 per (channel, quad);
    # fold the constant back in through the fc bias.
    b2_64 = conv2_b.astype(np.float64)
    corr = 2.0 * np.einsum(
        "cqj,c->j",
        Wfc7.reshape(7, 128, Q2).transpose(1, 2, 0) * 0.25, b2_64)
    b7 = b7 + corr
    b7rep = np.broadcast_to(
        b7.astype(np.float32)[None, None, :], (128, 4, 7)).copy()
    return {
        "W1p": W1p.astype(np.float16),
        "W2t": W2t.astype(np.float16),
        "W7t": W7t.astype(np.float16),
        "b2c": conv2_b.reshape(128, 1).astype(np.float32),
        "nb2": (-conv2_b).reshape(128, 1).astype(np.float32),
        "b7rep": np.ascontiguousarray(b7rep),
        "eye7": np.eye(7, dtype=np.float32),
    }


def prep_core_inputs(input_full, y0_full, core):
    base = core * BC
    inp = input_full[base:base + BC]
    inp_pad = np.zeros((BC, 5, L1 + 6), np.float32)
    inp_pad[:, :, 3:3 + L1] = inp
    X1 = np.empty((K1, L1, B2), np.float16)
    for h in range(2):
        samp = inp_pad[2 * np.arange(B2) + h]
        for ci in range(5):
            for k in range(7):
                X1[h * 35 + ci * 7 + k] = \
                    samp[:, ci, k:k + L1].T.astype(np.float16)
    X1[70] = 1.0
    # strip-major: [N_STRIPS, K1, NPOS_STRIP, B2], each strip contiguous
    X1 = np.ascontiguousarray(
        X1.reshape(K1, N_STRIPS, NPOS_STRIP, B2).transpose(1, 0, 2, 3))
    y0c = y0_full[base:base + BC]
    perm = np.concatenate([np.arange(0, BC, 2), np.arange(1, BC, 2)])
    y0dev = y0c[perm].astype(np.float32).reshape(4, 128).T.copy()
    return {"x1": X1, "y0c": np.ascontiguousarray(y0dev)}, perm


_CACHE = {}
LAST_RESULTS = None


def kernel(input, y0, conv1_w, conv1_b, conv2_w, conv2_b, fc_w, fc_b, L_w, L_b):
    key = "nc"
    if key not in _CACHE:
        weights = prep_weights(conv1_w, conv1_b, conv2_w, conv2_b,
                               fc_w, fc_b, L_w, L_b)
        _CACHE[key] = build_program(
            weights, repeat=int(os.environ.get("KERNEL_REPEAT", "1")))
    nc = _CACHE[key]

    in_maps = []
    perms = []
    for core in range(N_CORES):
        im, perm = prep_core_inputs(input, y0, core)
        in_maps.append(im)
        perms.append(perm)

    trace = bool(int(os.environ.get("KERNEL_TRACE", "0")))
    res = bass_utils.run_bass_kernel_spmd(
        nc, in_maps, core_ids=list(range(N_CORES)), trace=trace)
    global LAST_RESULTS
    LAST_RESULTS = res

    out = np.empty((B, T, 1), np.float32)
    for core in range(N_CORES):
        ydev = res.results[core]["out"].transpose(1, 0, 2).reshape(BC, T)
        base = core * BC
        out[base + perms[core], :, 0] = ydev
    return out


# revision 38
# speedup vs baseline: 1.1703x; 1.1048x over previous
"""Trainium2 Bass kernel for nn_NeuroScribe: CNN feature extractor + DMP integrator.

Strategy (per core, 512 samples, pure data-parallel across 8 cores):
  - Host folds L_w into fc_w (only 7 FC outputs needed: goal, w[5], tau),
    parity-packs samples (M = (out_ch, sample-parity) = 128), and builds the
    conv1 im2col (incl. a ones-row for the bias) in fp16.
  - conv1: 48 groups of 4 MMs (K=71, N=512) into one 4-bank PSUM tile
    (pe/po halves); relu+pool fused into eviction: ACT relu-evicts the po
    half, DVE scalar_tensor_tensor fuses relu(pe)+o_s, GPSIMD (mostly)
    folds the last pool level into h1.
  - conv2: two K=64 parity streams interleaved per-MM so they run in
    different PE row-groups concurrently (2x). 2-bank PSUM tiles (1 quad,
    10 MMs each). conv2 bias is NOT added on-chip: the eviction computes
    relu(x+b)-b via max(x,-b)+relu(po+b), and the constant offset is
    folded into the fc bias on the host.
  - fc: w7 [128,7] slices as stationary operand -> psum [7, 512]; 24
    accumulating MMs; PE transposes (identity trick) deliver g7 [128,4,7].
  - DMP: closed form. B_Z = A_Z/4 => critically damped: the 2x2 transition
    is lam*I + N with N nilpotent. x_t = d^t (geometric). All recurrences
    become tensor_tensor_scan ops; psi/fx evaluated for all t at once.
        y_t = lam^t y0 + t lam^(t-1) q1 + u S_t,  q1 = u(12.5 y0 + u)
        C_{t+1} = lam C_t + beta_t ; S_{t+1} = lam S_t + C_t
        beta_t = u (156.25 goal + fx_t)
"""
import os
import numpy as np

import concourse.bass as bass
import concourse.bacc as bacc
import concourse.mybir as mybir
from concourse import tile
from concourse import bass_utils

f32 = mybir.dt.float32
f16 = mybir.dt.float16
i32 = mybir.dt.int32
AF = mybir.ActivationFunctionType
ALU = mybir.AluOpType

N_CORES = 8
B = 4096
BC = B // N_CORES          # 512 samples per core
B2 = BC // 2               # 256 parity pairs
T = 101
NT = 100                   # scan steps
DT = 0.01
N_RBF = 5
_C = np.exp(-np.linspace(0.0, 1.0, N_RBF)).astype(np.float32)
_SIG2 = ((N_RBF ** 1.5) / _C).astype(np.float32)

L1 = 384                   # conv1 positions
Q1 = 96                    # pooled positions after pool1
Q2 = 24                    # pooled positions after pool2
K1 = 71                    # conv1 contraction (2 parities x 5ci x 7k + bias)
NPOS_STRIP = 64            # conv1 positions per X1 strip
N_STRIPS = L1 // NPOS_STRIP

# knobs: which conv pool-level-2 adds go to DVE vs GPSIMD
C1_L2_DVE_EVERY = 10 ** 9  # every Nth conv1 quad's l2-add goes to DVE
C2_L2_GS_EVERY = 4         # every Nth conv2 quad's l2-add goes to GPSIMD


def _dmp_prep(nc, dp):
    """g7-independent DMP constants; emitted early so they overlap conv."""
    st = {}
    st["ones"] = dp.tile([128, T], f32, name="ones")
    nc.vector.memset(st["ones"][:], 1.0)
    st["zeros"] = dp.tile([128, NT], f32, name="zeros")
    nc.vector.memset(st["zeros"][:], 0.0)
    tio = dp.tile([128, T], i32)
    nc.gpsimd.iota(tio[:], [[1, T]], base=0, channel_multiplier=0)
    st["tful"] = dp.tile([128, T], f32, name="tful")
    nc.vector.tensor_copy(st["tful"][:], tio[:])
    # only 10 physical [128,4,T] buffers; later phases alias tiles whose
    # earlier occupant is dead by then (saves ~10KB/partition of SBUF).
    for nm in ("xs", "lamt", "lamf_all", "x2", "den"):
        st[nm] = dp.tile([128, 4, T], f32, name=nm)
    st["psi"] = [dp.tile([128, 4, T], f32, name=f"psi{j}")
                 for j in range(N_RBF)]
    st["num"] = st["x2"]       # x2 dead once psi args built
    st["fx2"] = st["psi"][0]   # psi dead after the num chain
    st["beta"] = st["psi"][1]
    st["Cs"] = st["psi"][2]
    st["Ss"] = st["psi"][3]
    st["yout"] = st["psi"][4]
    st["sc"] = dp.tile([128, 12, 4], f32, name="sc")
    return st


def _emit_dmp(nc, dp, st, g7, y0t, outd):
    ones = st["ones"]
    zeros = st["zeros"]
    tful = st["tful"]
    xs = st["xs"]
    lamt = st["lamt"]
    lamf_all = st["lamf_all"]
    Cs = st["Cs"]
    Ss = st["Ss"]
    beta = st["beta"]
    num = st["num"]
    den = st["den"]
    fx2 = st["fx2"]
    yout = st["yout"]
    psi = st["psi"]

    def R2(t):
        return t.rearrange("p a b -> p (a b)")

    # ---- per-sample scalars, batched over the 4 sample chunks ----
    sc = st["sc"]
    tau = g7[:, :, 6]
    goal = g7[:, :, 0]
    u = sc[:, 0, :]
    lam = sc[:, 1, :]
    dg = sc[:, 2, :]
    kgy = sc[:, 3, :]
    q1l = sc[:, 4, :]
    bsc = sc[:, 5, :]
    bct = sc[:, 6, :]
    t0 = sc[:, 7, :]
    t1 = sc[:, 8, :]
    rl = sc[:, 9, :]
    t2 = sc[:, 10, :]
    nc.vector.tensor_scalar_mul(u, tau, DT)
    nc.vector.tensor_scalar(lam, tau, -0.125, 1.0, ALU.mult, ALU.add)
    nc.vector.tensor_scalar(dg, tau, -0.01, 1.0, ALU.mult, ALU.add)
    nc.vector.tensor_sub(kgy, goal, y0t[:])
    nc.vector.scalar_tensor_tensor(t0, y0t[:], 12.5, u, ALU.mult, ALU.add)
    nc.vector.tensor_mul(t1, u, t0)
    nc.vector.reciprocal(rl, lam)
    nc.vector.tensor_mul(q1l, t1, rl)
    nc.vector.tensor_mul(bsc, u, kgy)
    nc.vector.tensor_mul(t2, tau, goal)
    nc.vector.tensor_scalar_mul(bct, t2, 1.5625)

    for c in range(4):
        lamf = lamf_all[:, c, :]
        nc.vector.tensor_scalar_mul(lamf, ones[:], sc[:, 1, c:c + 1])
        dgf = beta[:, c, :]   # scratch
        nc.vector.tensor_scalar_mul(dgf, ones[:], sc[:, 2, c:c + 1])
        nc.vector.memset(xs[:, c, 0:1], 1.0)
        nc.vector.tensor_tensor_scan(
            xs[:, c, 1:T], dgf[:, 0:NT], zeros[:], 1.0,
            ALU.mult, ALU.add)
        nc.vector.memset(lamt[:, c, 0:1], 1.0)
        nc.vector.tensor_tensor_scan(
            lamt[:, c, 1:T], lamf[:, 0:NT], zeros[:], 1.0,
            ALU.mult, ALU.add)

    # psi_j = exp(a_j x^2 + b_j x + d_j): one shared Square, then per-j
    # affine (DVE/GS) + Exp (ACT) — shorter ACT chain than Square+Exp per j.
    x2 = st["x2"]
    nc.scalar.activation(R2(x2), R2(xs), AF.Square)
    for j in range(N_RBF):
        a_j = float(-0.5 / _SIG2[j])
        b_j = float(_C[j] / _SIG2[j])
        d_j = float(-0.5 * _C[j] * _C[j] / _SIG2[j])
        nc.vector.tensor_scalar(R2(psi[j]), R2(x2), a_j, d_j,
                                ALU.mult, ALU.add)
        nc.vector.scalar_tensor_tensor(R2(psi[j]), R2(xs), b_j,
                                       R2(psi[j]), ALU.mult, ALU.add)
        nc.scalar.activation(R2(psi[j]), R2(psi[j]), AF.Exp)
    nc.gpsimd.tensor_add(R2(den), R2(psi[0]), R2(psi[1]))
    nc.gpsimd.tensor_add(R2(fx2), R2(psi[2]), R2(psi[3]))
    nc.gpsimd.tensor_add(R2(den), R2(den), R2(fx2))
    nc.gpsimd.tensor_add(R2(den), R2(den), R2(psi[4]))
    nc.vector.reciprocal(R2(den), R2(den))

    for c in range(4):
        ncol = num[:, c, :]
        nc.vector.tensor_scalar_mul(ncol, psi[0][:, c, :], g7[:, c, 1:2])
        for j in range(1, N_RBF):
            nc.vector.scalar_tensor_tensor(
                ncol, psi[j][:, c, :], g7[:, c, 1 + j:2 + j],
                ncol, ALU.mult, ALU.add)
    nc.vector.tensor_mul(R2(fx2), R2(num), R2(den))
    nc.vector.tensor_mul(R2(fx2), R2(fx2), R2(xs))

    for c in range(4):
        uc = sc[:, 0, c:c + 1]
        q1c = sc[:, 4, c:c + 1]
        bscc = sc[:, 5, c:c + 1]
        bctc = sc[:, 6, c:c + 1]
        y0c = y0t[:, c:c + 1]
        nc.vector.tensor_scalar(beta[:, c, :], fx2[:, c, :],
                                bscc, bctc, ALU.mult, ALU.add)
        nc.vector.memset(Cs[:, c, 0:1], 0.0)
        nc.vector.tensor_tensor_scan(
            Cs[:, c, 1:T], lamf_all[:, c, 0:NT],
            beta[:, c, 0:NT], 0.0, ALU.mult, ALU.add)
        nc.vector.memset(Ss[:, c, 0:1], 0.0)
        nc.vector.tensor_tensor_scan(
            Ss[:, c, 1:T], lamf_all[:, c, 0:NT],
            Cs[:, c, 0:NT], 0.0, ALU.mult, ALU.add)
        # y = lamt*(y0 + t*q1l) + u*S
        a1 = num[:, c, :]
        nc.vector.tensor_scalar(a1, tful[:], q1c, y0c, ALU.mult, ALU.add)
        b1 = den[:, c, :]
        nc.gpsimd.tensor_mul(b1, lamt[:, c, :], a1)
        nc.vector.scalar_tensor_tensor(
            yout[:, c, :], Ss[:, c, :], uc, b1, ALU.mult, ALU.add)
        nc.sync.dma_start(outd[0:64, c, :], yout[0:64, c, :])
        nc.scalar.dma_start(outd[64:128, c, :], yout[64:128, c, :])


def build_program(weights, repeat=1):
    nc = bacc.Bacc(None, target_bir_lowering=False, debug=True)

    # strip-major so each strip is one contiguous DRAM region; every DMA
    # instruction stays <= 64 descriptors (else it degrades to one engine).
    x1d = nc.dram_tensor("x1", [N_STRIPS, K1, NPOS_STRIP, B2], f16,
                         kind="ExternalInput")
    y0d = nc.dram_tensor("y0c", [128, 4], f32, kind="ExternalInput")
    outd = nc.dram_tensor("out", [128, 4, T], f32, kind="ExternalOutput")

    w1d = nc.inline_tensor(weights["W1p"], "W1p")       # [K1, 128] f16
    w2d = nc.inline_tensor(weights["W2t"], "W2t")       # [128, 5, 128] f16
    w7d = nc.inline_tensor(weights["W7t"], "W7t")       # [128, 24, 7] f16
    b2d = nc.inline_tensor(weights["b2c"], "b2c")       # [128, 1] f32
    nb2d = nc.inline_tensor(weights["nb2"], "nb2")      # [128, 1] f32
    b7d = nc.inline_tensor(weights["b7rep"], "b7rep")   # [128, 4, 7] f32
    eyed = nc.inline_tensor(weights["eye7"], "eye7")    # [7, 7] f32

    with tile.TileContext(nc) as tc:
      for _rep in range(repeat):
        with tc.tile_pool(name="const", bufs=1) as cp, \
             tc.tile_pool(name="dmp", bufs=1) as dp, \
             tc.tile_pool(name="x1p", bufs=3) as xp:
            # strip prefetch first so the PE isn't blocked behind the
            # (latency-tolerant) weight loads on the two DMA rings.
            x1tiles = []

            def load_strip(s):
                x1t = xp.tile([128, NPOS_STRIP, B2], f16, tag="x1t",
                              name=f"x1t{s}")
                nc.sync.dma_start(x1t[0:36], x1d[s, 0:36])
                nc.scalar.dma_start(x1t[36:K1], x1d[s, 36:K1])
                x1tiles.append(x1t)

            # w1t + first strips split across both rings; all other weight
            # loads go on the sync ring only, so the scalar (ACT) queue is
            # free for PSUM evictions as soon as conv1 starts.
            w1t = cp.tile([K1, 128], f16)
            nc.sync.dma_start(w1t[0:36, :], w1d[0:36, :])
            nc.scalar.dma_start(w1t[36:K1, :], w1d[36:K1, :])
            load_strip(0)
            load_strip(1)
            w2t = cp.tile([128, 5, 128], f16)
            nc.sync.dma_start(w2t[0:64], w2d[0:64])
            nc.sync.dma_start(w2t[64:128], w2d[64:128])
            b2t = cp.tile([128, 1], f32)
            nc.sync.dma_start(b2t[0:64], b2d[0:64])
            nc.sync.dma_start(b2t[64:128], b2d[64:128])
            nb2t = cp.tile([128, 1], f32)
            nc.sync.dma_start(nb2t[0:64], nb2d[0:64])
            nc.sync.dma_start(nb2t[64:128], nb2d[64:128])
            w7t = cp.tile([128, 24, 7], f16)
            nc.sync.dma_start(w7t[0:64], w7d[0:64])
            nc.sync.dma_start(w7t[64:128], w7d[64:128])
            b7t = cp.tile([128, 4, 7], f32)
            nc.sync.dma_start(b7t[0:64], b7d[0:64])
            nc.sync.dma_start(b7t[64:128], b7d[64:128])
            eye7 = cp.tile([7, 7], f32)
            nc.sync.dma_start(eye7[:], eyed[:])
            y0t = cp.tile([128, 4], f32)
            nc.sync.dma_start(y0t[0:64], y0d[0:64])
            nc.sync.dma_start(y0t[64:128], y0d[64:128])
            dmp_st = _dmp_prep(nc, dp)

            # h1: [128=(par*64+ch), 100=(q in -2..97), 256] fp16, zero-padded
            h1 = cp.tile([128, Q1 + 4, B2], f16)
            nc.vector.memset(h1[:, 0:2, :], 0.0)
            nc.vector.memset(h1[:, Q1 + 2:Q1 + 4, :], 0.0)
            # h2p: [128=co2, 2=parity, 24=q4, 256] fp16
            h2p = cp.tile([128, 2, Q2, B2], f16, name="h2p")
            g7 = cp.tile([128, 4, 7], f32, name="g7")

            # ---------------- conv1 + conv2, interleaved ----------------
            # Quad-granular 2-bank PSUM tiles from one shared pool with
            # bufs=4 (8 banks): depth-4 pipelining hides the ~1.4us
            # eviction latency so the PE streams continuously and the HAM
            # clock-gate stays at full rate.
            # conv1 quad q1 (of 96): 2 MMs (pos pairs 2q1, 2q1+1).
            # conv2 quad q (of 24): 10 MMs per parity, parities
            # interleaved per-MM -> concurrent PE row-groups.
            with tc.tile_pool(name="ps", bufs=4, space="PSUM") as ps, \
                 tc.tile_pool(name="stg", bufs=3) as stp, \
                 tc.tile_pool(name="st2", bufs=3) as st2:

                def conv1_quad(q1):
                    s, lq = divmod(q1, NPOS_STRIP // 4)
                    if lq == 0 and s >= 2:
                        load_strip(s)
                    x1t = x1tiles[s]
                    pst = ps.tile([128, 2, 2, B2], f32, tag="ps",
                                  name=f"c1p{q1}")
                    for e in range(2):
                        m = lq * 2 + e
                        nc.tensor.matmul(
                            pst[:, e, :, :], w1t[:, :],
                            x1t[0:K1, 2 * m:2 * m + 2, :],
                            start=True, stop=True)
                    o_s = stp.tile([128, 2, B2], f16, tag="o_s",
                                   name=f"c1os{q1}")
                    nc.scalar.activation(o_s[:], pst[:, 1, :, :], AF.Relu)
                    l1t = stp.tile([128, 2, B2], f16, tag="l1t",
                                   name=f"c1l{q1}")
                    nc.vector.scalar_tensor_tensor(
                        l1t[:], pst[:, 0, :, :], 0.0, o_s[:],
                        ALU.max, ALU.add)
                    # l2: h1[2+q1] = l1t[0]+l1t[1]
                    eng = (nc.vector if (q1 % C1_L2_DVE_EVERY
                                         == C1_L2_DVE_EVERY - 1)
                           else nc.gpsimd)
                    eng.tensor_add(h1[:, 2 + q1, :],
                                   l1t[:, 0, :], l1t[:, 1, :])

                def conv2_quad_gen(q):
                    # generator: yields between MM chunks so conv1 quads
                    # interleave finely (keeps eviction demand smooth).
                    pstA = ps.tile([128, 2, 2, B2], f32, tag="ps",
                                   name=f"c2a{q}")
                    pstB = ps.tile([128, 2, 2, B2], f32, tag="ps",
                                   name=f"c2b{q}")
                    for kchunk in ((0, 1), (2, 3), (4,)):
                        for k in kchunk:
                            for i in range(2):
                                pp = 2 * q + i
                                nc.tensor.matmul(
                                    pstA[:, i, :, :], w2t[0:64, k, :],
                                    h1[0:64, 2 * pp + k:2 * pp + k + 2, :],
                                    start=(k == 0), stop=(k == 4))
                                nc.tensor.matmul(
                                    pstB[:, i, :, :], w2t[64:128, k, :],
                                    h1[64:128, 2 * pp + k:2 * pp + k + 2, :],
                                    start=(k == 0), stop=(k == 4))
                        yield
                    for par, pst2 in ((0, pstA), (1, pstB)):
                        o2 = st2.tile([128, 2, B2], f16, tag="o2",
                                      name=f"c2o{par}_{q}")
                        nc.scalar.activation(o2[:], pst2[:, 1, :, :],
                                             AF.Relu, bias=b2t[:, 0:1])
                        l2t = st2.tile([128, 2, B2], f16, tag="l2t",
                                       name=f"c2l{par}_{q}")
                        nc.vector.scalar_tensor_tensor(
                            l2t[:], pst2[:, 0, :, :], nb2t[:, 0:1], o2[:],
                            ALU.max, ALU.add)
                        eng = (nc.gpsimd if (q % C2_L2_GS_EVERY == 0)
                               else nc.vector)
                        eng.tensor_add(h2p[:, par, q, :],
                                       l2t[:, 0, :], l2t[:, 1, :])

                # conv2 quad q needs conv1 quads through 4q+5
                emitted = 0
                gen = None
                for q1 in range(96):
                    conv1_quad(q1)
                    if gen is not None and next(gen, "done") == "done":
                        gen = None
                    if (gen is None and emitted < Q2
                            and 4 * emitted + 5 <= q1):
                        gen = conv2_quad_gen(emitted)
                        emitted += 1
                        next(gen)
                if gen is not None:
                    for _ in gen:
                        pass
                while emitted < Q2:
                    for _ in conv2_quad_gen(emitted):
                        pass
                    emitted += 1

            # ---------------- fc: g7 = [samples, 7] ----------------
            with tc.tile_pool(name="psg", bufs=1, space="PSUM") as psg, \
                 tc.tile_pool(name="fst", bufs=1) as fst:
                pg = psg.tile([7, 2, B2], f32, tag="pg")
                for q4 in range(Q2):
                    nc.tensor.matmul(
                        pg[:], w7t[:, q4, :], h2p[:, :, q4, :],
                        start=(q4 == 0), stop=(q4 == Q2 - 1))
                g7s = fst.tile([7, 2, B2], f32)
                nc.vector.tensor_copy(g7s[:], pg[:])
                pgT = psg.tile([128, 4, 7], f32, tag="pgT")
                for c in range(4):
                    par, half = c // 2, c % 2
                    nc.tensor.transpose(
                        pgT[:, c, :],
                        g7s[:, par, half * 128:half * 128 + 128],
                        eye7[:])
                nc.vector.tensor_add(g7[:], pgT[:], b7t[:])

            # ---------------- DMP closed form ----------------
            _emit_dmp(nc, dp, dmp_st, g7, y0t, outd)

    nc.compile()
    return nc


# --------------------------------------------------------------------------
# host-side prep
# --------------------------------------------------------------------------

def prep_weights(conv1_w, conv1_b, conv2_w, conv2_b, fc_w, fc_b, L_w, L_b):
    W1p = np.zeros((K1, 128), np.float32)
    for h in range(2):
        W1p[h * 35:(h + 1) * 35, h * 64:h * 64 + 64] = \
            conv1_w.reshape(64, 35).T
        W1p[70, h * 64:h * 64 + 64] = conv1_b
    W2t = np.zeros((128, 5, 128), np.float32)
    for k in range(5):
        W2t[0:64, k, :] = conv2_w[:, :, k].T * 0.25
        W2t[64:128, k, :] = conv2_w[:, :, k].T * 0.25
    Wfc7 = np.concatenate(
        [fc_w[0:6].astype(np.float64),
         (L_w.astype(np.float64) @ fc_w.astype(np.float64))], axis=0)
    W7t = np.zeros((128, Q2, 7), np.float32)
    for j in range(7):
        W7t[:, :, j] = Wfc7[j].reshape(128, Q2) * 0.25
    b7 = np.concatenate(
        [fc_b[0:6].astype(np.float64),
         L_w.astype(np.float64) @ fc_b.astype(np.float64)
         + L_b.astype(np.float64)])
    # the on-chip h2p is sum(relu(conv2+b2)) - 2*b2 per (channel, quad);
    # fold the constant back in through the fc bias.
    b2_64 = conv2_b.astype(np.float64)
    corr = 2.0 * np.einsum(
        "cqj,c->j",
        Wfc7.reshape(7, 128, Q2).transpose(1, 2, 0) * 0.25, b2_64)
    b7 = b7 + corr
    b7rep = np.broadcast_to(
        b7.astype(np.float32)[None, None, :], (128, 4, 7)).copy()
    return {
        "W1p": W1p.astype(np.float16),
        "W2t": W2t.astype(np.float16),
        "W7t": W7t.astype(np.float16),
        "b2c": conv2_b.reshape(128, 1).astype(np.float32),
        "nb2": (-conv2_b).reshape(128, 1).astype(np.float32),
        "b7rep": np.ascontiguousarray(b7rep),
        "eye7": np.eye(7, dtype=np.float32),
    }


def prep_core_inputs(input_full, y0_full, core):
    base = core * BC
    inp = input_full[base:base + BC]
    inp_pad = np.zeros((BC, 5, L1 + 6), np.float32)
    inp_pad[:, :, 3:3 + L1] = inp
    X1 = np.empty((K1, L1, B2), np.float16)
    for h in range(2):
        samp = inp_pad[2 * np.arange(B2) + h]
        for ci in range(5):
            for k in range(7):
                X1[h * 35 + ci * 7 + k] = \
                    samp[:, ci, k:k + L1].T.astype(np.float16)
    X1[70] = 1.0
    # strip-major: [N_STRIPS, K1, NPOS_STRIP, B2], each strip contiguous
    X1 = np.ascontiguousarray(
        X1.reshape(K1, N_STRIPS, NPOS_STRIP, B2).transpose(1, 0, 2, 3))
    y0c = y0_full[base:base + BC]
    perm = np.concatenate([np.arange(0, BC, 2), np.arange(1, BC, 2)])
    y0dev = y0c[perm].astype(np.float32).reshape(4, 128).T.copy()
    return {"x1": X1, "y0c": np.ascontiguousarray(y0dev)}, perm


_CACHE = {}
LAST_RESULTS = None


def kernel(input, y0, conv1_w, conv1_b, conv2_w, conv2_b, fc_w, fc_b, L_w, L_b):
    key = "nc"
    if key not in _CACHE:
        weights = prep_weights(conv1_w, conv1_b, conv2_w, conv2_b,
                               fc_w, fc_b, L_w, L_b)
        _CACHE[key] = build_program(
            weights, repeat=int(os.environ.get("KERNEL_REPEAT", "1")))
    nc = _CACHE[key]

    in_maps = []
    perms = []
    for core in range(N_CORES):
        im, perm = prep_core_inputs(input, y0, core)
        in_maps.append(im)
        perms.append(perm)

    trace = bool(int(os.environ.get("KERNEL_TRACE", "0")))
    res = bass_utils.run_bass_kernel_spmd(
        nc, in_maps, core_ids=list(range(N_CORES)), trace=trace)
    global LAST_RESULTS
    LAST_RESULTS = res

    out = np.empty((B, T, 1), np.float32)
    for core in range(N_CORES):
        ydev = res.results[core]["out"].transpose(1, 0, 2).reshape(BC, T)
        base = core * BC
        out[base + perms[core], :, 0] = ydev
    return out


# revision 42
# speedup vs baseline: 1.1973x; 1.0230x over previous
"""Trainium2 Bass kernel for nn_NeuroScribe: CNN feature extractor + DMP integrator.

Strategy (per core, 512 samples, pure data-parallel across 8 cores):
  - Host folds L_w into fc_w (only 7 FC outputs needed: goal, w[5], tau),
    parity-packs samples (M = (out_ch, sample-parity) = 128), and builds the
    conv1 im2col (incl. a ones-row for the bias) in fp16.
  - conv1: 48 groups of 4 MMs (K=71, N=512) into one 4-bank PSUM tile
    (pe/po halves); relu+pool fused into eviction: ACT relu-evicts the po
    half, DVE scalar_tensor_tensor fuses relu(pe)+o_s, GPSIMD (mostly)
    folds the last pool level into h1.
  - conv2: two K=64 parity streams interleaved per-MM so they run in
    different PE row-groups concurrently (2x). 2-bank PSUM tiles (1 quad,
    10 MMs each). conv2 bias is NOT added on-chip: the eviction computes
    relu(x+b)-b via max(x,-b)+relu(po+b), and the constant offset is
    folded into the fc bias on the host.
  - fc: w7 [128,7] slices as stationary operand -> psum [7, 512]; 24
    accumulating MMs; PE transposes (identity trick) deliver g7 [128,4,7].
  - DMP: closed form. B_Z = A_Z/4 => critically damped: the 2x2 transition
    is lam*I + N with N nilpotent. x_t = d^t (geometric). All recurrences
    become tensor_tensor_scan ops; psi/fx evaluated for all t at once.
        y_t = lam^t y0 + t lam^(t-1) q1 + u S_t,  q1 = u(12.5 y0 + u)
        C_{t+1} = lam C_t + beta_t ; S_{t+1} = lam S_t + C_t
        beta_t = u (156.25 goal + fx_t)
"""
import os
import numpy as np

import concourse.bass as bass
import concourse.bacc as bacc
import concourse.mybir as mybir
from concourse import tile
from concourse import bass_utils

f32 = mybir.dt.float32
f16 = mybir.dt.float16
i32 = mybir.dt.int32
AF = mybir.ActivationFunctionType
ALU = mybir.AluOpType

N_CORES = 8
B = 4096
BC = B // N_CORES          # 512 samples per core
B2 = BC // 2               # 256 parity pairs
T = 101
NT = 100                   # scan steps
DT = 0.01
N_RBF = 5
_C = np.exp(-np.linspace(0.0, 1.0, N_RBF)).astype(np.float32)
_SIG2 = ((N_RBF ** 1.5) / _C).astype(np.float32)

L1 = 384                   # conv1 positions
Q1 = 96                    # pooled positions after pool1
Q2 = 24                    # pooled positions after pool2
K1 = 71                    # conv1 contraction (2 parities x 5ci x 7k + bias)
NPOS_STRIP = 64            # conv1 positions per X1 strip
N_STRIPS = L1 // NPOS_STRIP

# knobs: which conv pool-level-2 adds go to DVE vs GPSIMD
C1_L2_DVE_EVERY = 10 ** 9  # every Nth conv1 quad's l2-add goes to DVE
C2_L2_GS_EVERY = 4         # every Nth conv2 quad's l2-add goes to GPSIMD


def _dmp_prep(nc, dp):
    """g7-independent DMP constants; emitted early so they overlap conv."""
    st = {}
    st["ones"] = dp.tile([128, T], f32, name="ones")
    nc.vector.memset(st["ones"][:], 1.0)
    st["zeros"] = dp.tile([128, NT], f32, name="zeros")
    nc.vector.memset(st["zeros"][:], 0.0)
    tio = dp.tile([128, T], i32)
    nc.gpsimd.iota(tio[:], [[1, T]], base=0, channel_multiplier=0)
    st["tful"] = dp.tile([128, T], f32, name="tful")
    nc.vector.tensor_copy(st["tful"][:], tio[:])
    # psi/x2/num in f16 (DVE 2x/4x modes + smaller); yout aliases
    # lamf_all whose per-chunk lifetime ends before yout's writes.
    for nm in ("xs", "lamt", "lamf_all", "den", "fx2", "beta",
               "Cs", "Ss"):
        st[nm] = dp.tile([128, 4, T], f32, name=nm)
    st["psi"] = [dp.tile([128, 4, T], f16, name=f"psi{j}")
                 for j in range(N_RBF)]
    st["x2"] = dp.tile([128, 4, T], f16, name="x2")
    st["num"] = dp.tile([128, 4, T], f16, name="numt")
    st["yout"] = st["lamf_all"]
    st["sc"] = dp.tile([128, 12, 4], f32, name="sc")
    return st


def _emit_dmp(nc, dp, st, g7, y0t, outd):
    ones = st["ones"]
    zeros = st["zeros"]
    tful = st["tful"]
    xs = st["xs"]
    lamt = st["lamt"]
    lamf_all = st["lamf_all"]
    Cs = st["Cs"]
    Ss = st["Ss"]
    beta = st["beta"]
    num = st["num"]
    den = st["den"]
    fx2 = st["fx2"]
    yout = st["yout"]
    psi = st["psi"]

    def R2(t):
        return t.rearrange("p a b -> p (a b)")

    # ---- per-sample scalars, batched over the 4 sample chunks ----
    sc = st["sc"]
    tau = g7[:, :, 6]
    goal = g7[:, :, 0]
    u = sc[:, 0, :]
    lam = sc[:, 1, :]
    dg = sc[:, 2, :]
    kgy = sc[:, 3, :]
    q1l = sc[:, 4, :]
    bsc = sc[:, 5, :]
    bct = sc[:, 6, :]
    t0 = sc[:, 7, :]
    t1 = sc[:, 8, :]
    rl = sc[:, 9, :]
    t2 = sc[:, 10, :]
    nc.vector.tensor_scalar_mul(u, tau, DT)
    nc.vector.tensor_scalar(lam, tau, -0.125, 1.0, ALU.mult, ALU.add)
    nc.vector.tensor_scalar(dg, tau, -0.01, 1.0, ALU.mult, ALU.add)
    nc.vector.tensor_sub(kgy, goal, y0t[:])
    nc.vector.scalar_tensor_tensor(t0, y0t[:], 12.5, u, ALU.mult, ALU.add)
    nc.vector.tensor_mul(t1, u, t0)
    nc.vector.reciprocal(rl, lam)
    nc.vector.tensor_mul(q1l, t1, rl)
    nc.vector.tensor_mul(bsc, u, kgy)
    nc.vector.tensor_mul(t2, tau, goal)
    nc.vector.tensor_scalar_mul(bct, t2, 1.5625)

    for c in range(4):
        lamf = lamf_all[:, c, :]
        nc.vector.tensor_scalar_mul(lamf, ones[:], sc[:, 1, c:c + 1])
        dgf = beta[:, c, :]   # scratch
        nc.vector.tensor_scalar_mul(dgf, ones[:], sc[:, 2, c:c + 1])
        nc.vector.memset(xs[:, c, 0:1], 1.0)
        nc.vector.tensor_tensor_scan(
            xs[:, c, 1:T], dgf[:, 0:NT], zeros[:], 1.0,
            ALU.mult, ALU.add)
        nc.vector.memset(lamt[:, c, 0:1], 1.0)
        nc.vector.tensor_tensor_scan(
            lamt[:, c, 1:T], lamf[:, 0:NT], zeros[:], 1.0,
            ALU.mult, ALU.add)

    # psi_j = exp(a_j x^2 + b_j x + d_j): one shared Square, then per-j
    # affine (DVE/GS) + Exp (ACT) — shorter ACT chain than Square+Exp per j.
    x2 = st["x2"]
    nc.scalar.activation(R2(x2), R2(xs), AF.Square)
    for j in range(N_RBF):
        a_j = float(-0.5 / _SIG2[j])
        b_j = float(_C[j] / _SIG2[j])
        d_j = float(-0.5 * _C[j] * _C[j] / _SIG2[j])
        nc.vector.tensor_scalar(R2(psi[j]), R2(x2), a_j, d_j,
                                ALU.mult, ALU.add)
        nc.vector.scalar_tensor_tensor(R2(psi[j]), R2(xs), b_j,
                                       R2(psi[j]), ALU.mult, ALU.add)
        nc.scalar.activation(R2(psi[j]), R2(psi[j]), AF.Exp)
    nc.gpsimd.tensor_add(R2(den), R2(psi[0]), R2(psi[1]))
    nc.gpsimd.tensor_add(R2(fx2), R2(psi[2]), R2(psi[3]))
    nc.gpsimd.tensor_add(R2(den), R2(den), R2(fx2))
    nc.gpsimd.tensor_add(R2(den), R2(den), R2(psi[4]))
    nc.vector.reciprocal(R2(den), R2(den))

    for c in range(4):
        ncol = num[:, c, :]
        nc.vector.tensor_scalar_mul(ncol, psi[0][:, c, :], g7[:, c, 1:2])
        for j in range(1, N_RBF):
            nc.vector.scalar_tensor_tensor(
                ncol, psi[j][:, c, :], g7[:, c, 1 + j:2 + j],
                ncol, ALU.mult, ALU.add)
    nc.vector.tensor_mul(R2(fx2), R2(num), R2(den))
    nc.vector.tensor_mul(R2(fx2), R2(fx2), R2(xs))

    for c in range(4):
        uc = sc[:, 0, c:c + 1]
        q1c = sc[:, 4, c:c + 1]
        bscc = sc[:, 5, c:c + 1]
        bctc = sc[:, 6, c:c + 1]
        y0c = y0t[:, c:c + 1]
        nc.vector.tensor_scalar(beta[:, c, :], fx2[:, c, :],
                                bscc, bctc, ALU.mult, ALU.add)
        nc.vector.memset(Cs[:, c, 0:1], 0.0)
        nc.vector.tensor_tensor_scan(
            Cs[:, c, 1:T], lamf_all[:, c, 0:NT],
            beta[:, c, 0:NT], 0.0, ALU.mult, ALU.add)
        nc.vector.memset(Ss[:, c, 0:1], 0.0)
        nc.vector.tensor_tensor_scan(
            Ss[:, c, 1:T], lamf_all[:, c, 0:NT],
            Cs[:, c, 0:NT], 0.0, ALU.mult, ALU.add)
        # y = lamt*(y0 + t*q1l) + u*S
        a1 = num[:, c, :]
        nc.vector.tensor_scalar(a1, tful[:], q1c, y0c, ALU.mult, ALU.add)
        b1 = den[:, c, :]
        nc.gpsimd.tensor_mul(b1, lamt[:, c, :], a1)
        nc.vector.scalar_tensor_tensor(
            yout[:, c, :], Ss[:, c, :], uc, b1, ALU.mult, ALU.add)
        nc.sync.dma_start(outd[0:64, c, :], yout[0:64, c, :])
        nc.scalar.dma_start(outd[64:128, c, :], yout[64:128, c, :])


def build_program(weights, repeat=1):
    nc = bacc.Bacc(None, target_bir_lowering=False, debug=True)

    # strip-major so each strip is one contiguous DRAM region; every DMA
    # instruction stays <= 64 descriptors (else it degrades to one engine).
    x1d = nc.dram_tensor("x1", [N_STRIPS, K1, NPOS_STRIP, B2], f16,
                         kind="ExternalInput")
    y0d = nc.dram_tensor("y0c", [128, 4], f32, kind="ExternalInput")
    outd = nc.dram_tensor("out", [128, 4, T], f32, kind="ExternalOutput")

    w1d = nc.inline_tensor(weights["W1p"], "W1p")       # [K1, 128] f16
    w2d = nc.inline_tensor(weights["W2t"], "W2t")       # [128, 5, 128] f16
    w7d = nc.inline_tensor(weights["W7t"], "W7t")       # [128, 24, 7] f16
    b2d = nc.inline_tensor(weights["b2c"], "b2c")       # [128, 1] f32
    nb2d = nc.inline_tensor(weights["nb2"], "nb2")      # [128, 1] f32
    b7d = nc.inline_tensor(weights["b7rep"], "b7rep")   # [128, 4, 7] f32
    eyed = nc.inline_tensor(weights["eye7"], "eye7")    # [7, 7] f32

    with tile.TileContext(nc) as tc:
      for _rep in range(repeat):
        with tc.tile_pool(name="const", bufs=1) as cp, \
             tc.tile_pool(name="dmp", bufs=1) as dp, \
             tc.tile_pool(name="x1p", bufs=3) as xp:
            # strip prefetch first so the PE isn't blocked behind the
            # (latency-tolerant) weight loads on the two DMA rings.
            x1tiles = []

            def load_strip(s, head_split=False):
                x1t = xp.tile([128, NPOS_STRIP, B2], f16, tag="x1t",
                              name=f"x1t{s}")
                if head_split:
                    # first 16 positions land first so conv1 can start
                    # while the bulk of the strip is still in flight
                    nc.sync.dma_start(x1t[0:36, 0:16], x1d[s, 0:36, 0:16])
                    nc.scalar.dma_start(x1t[36:K1, 0:16],
                                        x1d[s, 36:K1, 0:16])
                    nc.sync.dma_start(x1t[0:36, 16:], x1d[s, 0:36, 16:])
                    nc.scalar.dma_start(x1t[36:K1, 16:],
                                        x1d[s, 36:K1, 16:])
                else:
                    nc.sync.dma_start(x1t[0:36], x1d[s, 0:36])
                    nc.scalar.dma_start(x1t[36:K1], x1d[s, 36:K1])
                x1tiles.append(x1t)

            # w1t + first strips split across both rings; all other weight
            # loads go on the sync ring only, so the scalar (ACT) queue is
            # free for PSUM evictions as soon as conv1 starts.
            w1t = cp.tile([K1, 128], f16)
            nc.sync.dma_start(w1t[0:36, :], w1d[0:36, :])
            nc.scalar.dma_start(w1t[36:K1, :], w1d[36:K1, :])
            load_strip(0, head_split=True)
            load_strip(1)
            w2t = cp.tile([128, 5, 128], f16)
            nc.sync.dma_start(w2t[0:64], w2d[0:64])
            nc.sync.dma_start(w2t[64:128], w2d[64:128])
            b2t = cp.tile([128, 1], f32)
            nc.sync.dma_start(b2t[0:64], b2d[0:64])
            nc.sync.dma_start(b2t[64:128], b2d[64:128])
            nb2t = cp.tile([128, 1], f32)
            nc.sync.dma_start(nb2t[0:64], nb2d[0:64])
            nc.sync.dma_start(nb2t[64:128], nb2d[64:128])
            w7t = cp.tile([128, 24, 7], f16)
            nc.sync.dma_start(w7t[0:64], w7d[0:64])
            nc.sync.dma_start(w7t[64:128], w7d[64:128])
            b7t = cp.tile([128, 4, 7], f32)
            nc.sync.dma_start(b7t[0:64], b7d[0:64])
            nc.sync.dma_start(b7t[64:128], b7d[64:128])
            eye7 = cp.tile([7, 7], f32)
            nc.sync.dma_start(eye7[:], eyed[:])
            y0t = cp.tile([128, 4], f32)
            nc.sync.dma_start(y0t[0:64], y0d[0:64])
            nc.sync.dma_start(y0t[64:128], y0d[64:128])
            dmp_st = _dmp_prep(nc, dp)

            # h1: [128=(par*64+ch), 100=(q in -2..97), 256] fp16, zero-padded
            h1 = cp.tile([128, Q1 + 4, B2], f16)
            nc.vector.memset(h1[:, 0:2, :], 0.0)
            nc.vector.memset(h1[:, Q1 + 2:Q1 + 4, :], 0.0)
            # h2p: [128=co2, 2=parity, 24=q4, 256] fp16
            h2p = cp.tile([128, 2, Q2, B2], f16, name="h2p")
            g7 = cp.tile([128, 4, 7], f32, name="g7")

            # ---------------- conv1 + conv2, interleaved ----------------
            # Quad-granular 2-bank PSUM tiles from one shared pool with
            # bufs=4 (8 banks): depth-4 pipelining hides the ~1.4us
            # eviction latency so the PE streams continuously and the HAM
            # clock-gate stays at full rate.
            # conv1 quad q1 (of 96): 2 MMs (pos pairs 2q1, 2q1+1).
            # conv2 quad q (of 24): 10 MMs per parity, parities
            # interleaved per-MM -> concurrent PE row-groups.
            with tc.tile_pool(name="ps", bufs=4, space="PSUM") as ps, \
                 tc.tile_pool(name="stg", bufs=4) as stp, \
                 tc.tile_pool(name="st2", bufs=3) as st2:

                def conv1_quad(q1):
                    s, lq = divmod(q1, NPOS_STRIP // 4)
                    if lq == 0 and s >= 2:
                        load_strip(s)
                    x1t = x1tiles[s]
                    pst = ps.tile([128, 2, 2, B2], f32, tag="ps",
                                  name=f"c1p{q1}")
                    for e in range(2):
                        m = lq * 2 + e
                        nc.tensor.matmul(
                            pst[:, e, :, :], w1t[:, :],
                            x1t[0:K1, 2 * m:2 * m + 2, :],
                            start=True, stop=True)
                    o_s = stp.tile([128, 2, B2], f16, tag="o_s",
                                   name=f"c1os{q1}")
                    nc.scalar.activation(o_s[:], pst[:, 1, :, :], AF.Relu)
                    l1t = stp.tile([128, 2, B2], f16, tag="l1t",
                                   name=f"c1l{q1}")
                    nc.vector.scalar_tensor_tensor(
                        l1t[:], pst[:, 0, :, :], 0.0, o_s[:],
                        ALU.max, ALU.add)
                    # l2: h1[2+q1] = l1t[0]+l1t[1]
                    eng = (nc.vector if (q1 % C1_L2_DVE_EVERY
                                         == C1_L2_DVE_EVERY - 1)
                           else nc.gpsimd)
                    eng.tensor_add(h1[:, 2 + q1, :],
                                   l1t[:, 0, :], l1t[:, 1, :])

                def conv2_quad_gen(q):
                    # generator: yields between MM chunks so conv1 quads
                    # interleave finely (keeps eviction demand smooth).
                    pstA = ps.tile([128, 2, 2, B2], f32, tag="ps",
                                   name=f"c2a{q}")
                    pstB = ps.tile([128, 2, 2, B2], f32, tag="ps",
                                   name=f"c2b{q}")
                    for kchunk in ((0, 1), (2, 3), (4,)):
                        for k in kchunk:
                            for i in range(2):
                                pp = 2 * q + i
                                nc.tensor.matmul(
                                    pstA[:, i, :, :], w2t[0:64, k, :],
                                    h1[0:64, 2 * pp + k:2 * pp + k + 2, :],
                                    start=(k == 0), stop=(k == 4))
                                nc.tensor.matmul(
                                    pstB[:, i, :, :], w2t[64:128, k, :],
                                    h1[64:128, 2 * pp + k:2 * pp + k + 2, :],
                                    start=(k == 0), stop=(k == 4))
                        yield
                    for par, pst2 in ((0, pstA), (1, pstB)):
                        o2 = st2.tile([128, 2, B2], f16, tag="o2",
                                      name=f"c2o{par}_{q}")
                        nc.scalar.activation(o2[:], pst2[:, 1, :, :],
                                             AF.Relu, bias=b2t[:, 0:1])
                        l2t = st2.tile([128, 2, B2], f16, tag="l2t",
                                       name=f"c2l{par}_{q}")
                        nc.vector.scalar_tensor_tensor(
                            l2t[:], pst2[:, 0, :, :], nb2t[:, 0:1], o2[:],
                            ALU.max, ALU.add)
                        eng = (nc.gpsimd if (q % C2_L2_GS_EVERY == 0)
                               else nc.vector)
                        eng.tensor_add(h2p[:, par, q, :],
                                       l2t[:, 0, :], l2t[:, 1, :])

                # conv2 quad q needs conv1 quads through 4q+5
                emitted = 0
                gen = None
                for q1 in range(96):
                    conv1_quad(q1)
                    if gen is not None and next(gen, "done") == "done":
                        gen = None
                    if (gen is None and emitted < Q2
                            and 4 * emitted + 5 <= q1):
                        gen = conv2_quad_gen(emitted)
                        emitted += 1
                        next(gen)
                if gen is not None:
                    for _ in gen:
                        pass
                while emitted < Q2:
                    for _ in conv2_quad_gen(emitted):
                        pass
                    emitted += 1

            # ---------------- fc: g7 = [samples, 7] ----------------
            with tc.tile_pool(name="psg", bufs=1, space="PSUM") as psg, \
                 tc.tile_pool(name="fst", bufs=1) as fst:
                pg = psg.tile([7, 2, B2], f32, tag="pg")
                for q4 in range(Q2):
                    nc.tensor.matmul(
                        pg[:], w7t[:, q4, :], h2p[:, :, q4, :],
                        start=(q4 == 0), stop=(q4 == Q2 - 1))
                g7s = fst.tile([7, 2, B2], f32)
                nc.vector.tensor_copy(g7s[:], pg[:])
                pgT = psg.tile([128, 4, 7], f32, tag="pgT")
                for c in range(4):
                    par, half = c // 2, c % 2
                    nc.tensor.transpose(
                        pgT[:, c, :],
                        g7s[:, par, half * 128:half * 128 + 128],
                        eye7[:])
                nc.vector.tensor_add(g7[:], pgT[:], b7t[:])

            # ---------------- DMP closed form ----------------
            _emit_dmp(nc, dp, dmp_st, g7, y0t, outd)

    nc.compile()
    return nc


# --------------------------------------------------------------------------
# host-side prep
# --------------------------------------------------------------------------

def prep_weights(conv1_w, conv1_b, conv2_w, conv2_b, fc_w, fc_b, L_w, L_b):
    W1p = np.zeros((K1, 128), np.float32)
    for h in range(2):
        W1p[h * 35:(h + 1) * 35, h * 64:h * 64 + 64] = \
            conv1_w.reshape(64, 35).T
        W1p[70, h * 64:h * 64 + 64] = conv1_b
    W2t = np.zeros((128, 5, 128), np.float32)
    for k in range(5):
        W2t[0:64, k, :] = conv2_w[:, :, k].T * 0.25
        W2t[64:128, k, :] = conv2_w[:, :, k].T * 0.25
    Wfc7 = np.concatenate(
        [fc_w[0:6].astype(np.float64),
         (L_w.astype(np.float64) @ fc_w.astype(np.float64))], axis=0)
    W7t = np.zeros((128, Q2, 7), np.float32)
    for j in range(7):
        W7t[:, :, j] = Wfc7[j].reshape(128, Q2) * 0.25
    b7 = np.concatenate(
        [fc_b[0:6].astype(np.float64),
         L_w.astype(np.float64) @ fc_b.astype(np.float64)
         + L_b.astype(np.float64)])
    # the on-chip h2p is sum(relu(conv2+b2)) - 2*b2 per (channel, quad);
    # fold the constant back in through the fc bias.
    b2_64 = conv2_b.astype(np.float64)
    corr = 2.0 * np.einsum(
        "cqj,c->j",
        Wfc7.reshape(7, 128, Q2).transpose(1, 2, 0) * 0.25, b2_64)
    b7 = b7 + corr
    b7rep = np.broadcast_to(
        b7.astype(np.float32)[None, None, :], (128, 4, 7)).copy()
    return {
        "W1p": W1p.astype(np.float16),
        "W2t": W2t.astype(np.float16),
        "W7t": W7t.astype(np.float16),
        "b2c": conv2_b.reshape(128, 1).astype(np.float32),
        "nb2": (-conv2_b).reshape(128, 1).astype(np.float32),
        "b7rep": np.ascontiguousarray(b7rep),
        "eye7": np.eye(7, dtype=np.float32),
    }


def prep_core_inputs(input_full, y0_full, core):
    base = core * BC
    inp = input_full[base:base + BC]
    inp_pad = np.zeros((BC, 5, L1 + 6), np.float32)
    inp_pad[:, :, 3:3 + L1] = inp
    X1 = np.empty((K1, L1, B2), np.float16)
    for h in range(2):
        samp = inp_pad[2 * np.arange(B2) + h]
        for ci in range(5):
            for k in range(7):
                X1[h * 35 + ci * 7 + k] = \
                    samp[:, ci, k:k + L1].T.astype(np.float16)
    X1[70] = 1.0
    # strip-major: [N_STRIPS, K1, NPOS_STRIP, B2], each strip contiguous
    X1 = np.ascontiguousarray(
        X1.reshape(K1, N_STRIPS, NPOS_STRIP, B2).transpose(1, 0, 2, 3))
    y0c = y0_full[base:base + BC]
    perm = np.concatenate([np.arange(0, BC, 2), np.arange(1, BC, 2)])
    y0dev = y0c[perm].astype(np.float32).reshape(4, 128).T.copy()
    return {"x1": X1, "y0c": np.ascontiguousarray(y0dev)}, perm


_CACHE = {}
LAST_RESULTS = None


def kernel(input, y0, conv1_w, conv1_b, conv2_w, conv2_b, fc_w, fc_b, L_w, L_b):
    key = "nc"
    if key not in _CACHE:
        weights = prep_weights(conv1_w, conv1_b, conv2_w, conv2_b,
                               fc_w, fc_b, L_w, L_b)
        _CACHE[key] = build_program(
            weights, repeat=int(os.environ.get("KERNEL_REPEAT", "1")))
    nc = _CACHE[key]

    in_maps = []
    perms = []
    for core in range(N_CORES):
        im, perm = prep_core_inputs(input, y0, core)
        in_maps.append(im)
        perms.append(perm)

    trace = bool(int(os.environ.get("KERNEL_TRACE", "0")))
    res = bass_utils.run_bass_kernel_spmd(
        nc, in_maps, core_ids=list(range(N_CORES)), trace=trace)
    global LAST_RESULTS
    LAST_RESULTS = res

    out = np.empty((B, T, 1), np.float32)
    for core in range(N_CORES):
        ydev = res.results[core]["out"].transpose(1, 0, 2).reshape(BC, T)
        base = core * BC
        out[base + perms[core], :, 0] = ydev
    return out
